# revision 12
# baseline (speedup 1.0000x reference)
"""Trainium2 Bass kernel for a single-step GRU attention decoder.

Math (matches the reference nn.Module):
    xe  = emb[x]                                   # [H]
    a   = log_softmax(cat(xe, h0) @ attn_W.T + attn_b)   # [L]
    ap  = a @ encoder_outputs                      # [H]
    g   = relu(cat(xe, ap) @ ctx_W.T + ctx_b)      # [H]
    GRU(g, h0) -> h_new                            # [H]
    logits = h_new @ out_W.T + out_b               # [V]

Distribution across 8 NeuronCores (one TRN2 chip):
  - attention sharded over L (512 rows/core); exploiting linearity,
    log_softmax @ enc == a @ enc - (log sum exp a) * colsum(enc), so one
    collective of per-core partials {a@enc, colsum(enc), sum(exp a)} lets
    every core reconstruct attn_applied locally.
  - ctx projection sharded by output rows; GRU mats sharded by *input*
    columns so a single collective of partial (gi, gh) lets every core
    compute the full gates / h_new locally.
  - out projection sharded over vocab (6283 rows/core, padded to 6400).

Both collectives are AllGathers (≈4.6us floor vs ≈10us for AllReduce on
8 cores) followed by a 3-level on-chip fold (log2(8) tensor_adds).

Precision: bulk weights are bf16; "lo" residual terms (enc, ctx ap-part,
W_ih) and the *whole* of W_hh / out_W are fp8 e3m4 with power-of-2
scales folded into the matmul rhs (exact in bf16), halving out_W HBM
traffic.  fp32 accumulation in PSUM throughout; gates/softmax in fp32.
End-to-end rel err ~1.2e-2 (sim) vs the 2e-2 gate.

The PE idles during collective waits; dummy matmuls are issued in those
windows to keep the HAM clock-gate warm (PE at 2.4GHz, not 1.2GHz).
"""

import ml_dtypes
import numpy as np

import concourse.bass as bass
import concourse.bacc as bacc
import concourse.tile as tile
from concourse import mybir
from concourse.bass_utils import run_bass_kernel_spmd

H = 1024
V = 50257
L = 4096
NCORES = 8
LC = L // NCORES          # 512 encoder rows per core
HC = H // NCORES          # 128 hidden chunk per core
VC = -(-V // NCORES)      # 6283 vocab rows per core
VT = 50                   # vocab tiles of 128 per core
VPAD = VT * 128           # 6400
F32 = mybir.dt.float32
BF16 = mybir.dt.bfloat16
E3 = mybir.dt.float8e3
NPBF16 = ml_dtypes.bfloat16
NPE3 = ml_dtypes.float8_e3m4
RG = [list(range(NCORES))]

# power-of-2 scales for the e3m4 tensors (folded into rhs; exact in bf16)
S_ENC = 512.0
S_CTX = 2048.0
S_WIH = 2048.0
S_WHH = 256.0
S_OUT = 256.0
S_ATTN = 256.0

_CACHE = {}


def _build(dbg=False):
    key = ("nc", dbg)
    if key in _CACHE:
        return _CACHE[key]

    nc = bacc.Bacc("TRN2", target_bir_lowering=False, debug=False,
                   num_devices=NCORES)

    def din(name, shape, dt=F32):
        return nc.dram_tensor(name, shape, dt, kind="ExternalInput")

    catin_d = din("catin", [128, 16], BF16)    # cols 0-7 xe, 8-15 h0
    catins_d = din("catins", [128, 16], BF16)  # catin / S_ATTN (for e3 attn)
    attn_wt_d = din("attn_wt", [128, 16 * LC], E3)
    attn_b_d = din("attn_b", [128, 4])
    enc_hi_d = din("enc_hi", [128, 4 * H], BF16)
    enc_lo_d = din("enc_lo", [128, 4 * H], E3)
    ctx_hi_d = din("ctx_hi", [128, 2048], BF16)
    ctx_lo_d = din("ctx_lo", [128, 1024], E3)
    ctx_b_d = din("ctx_b", [128, 1])
    wih_hi_d = din("wih_hi", [HC, 3 * H], BF16)
    wih_lo_d = din("wih_lo", [HC, 3 * H], E3)
    whh_d = din("whh", [HC, 3 * H], E3)
    h0c_d = din("h0c", [128, 1], BF16)         # h0 chunk k / S_WHH
    h0cm_d = din("h0cm", [128, 8])             # full h0, col-major, fp32
    bias8_d = din("bias8", [128, 48])          # cat(b_ih, b_hh)/8 col-major
    out_wt_d = din("out_wt", [H, VPAD], E3)    # out_W vocab chunk, transposed
    out_b_d = din("out_b", [128, VT])
    out_d = nc.dram_tensor("out", [128, VT], F32, kind="ExternalOutput")

    AG = "AllGather"
    BYP = mybir.AluOpType.bypass
    ACTF = mybir.ActivationFunctionType

    with tile.TileContext(nc) as tc:
        with (
            tc.tile_pool(name="wp", bufs=1) as wp,
            tc.tile_pool(name="sp", bufs=1) as sp,
            tc.tile_pool(name="pp", bufs=1, space="PSUM") as pp,
            tc.tile_pool(name="dp", bufs=1, space="DRAM") as dp,
        ):
            # ------------- loads (issue order = priority order) -------------
            # small tiles go on the scalar HWDGE ring (qAct) so the sync
            # ring (qSP) starts streaming the big prefix immediately; both
            # rings' DMAs complete early so no sem-lane entanglement.
            catin = sp.tile([128, 16], BF16, tag="catin")
            nc.scalar.dma_start(catin[:], catin_d[:])
            catins = sp.tile([128, 16], BF16, tag="catins")
            nc.scalar.dma_start(catins[:], catins_d[:])
            abias = sp.tile([128, 4], F32, tag="abias")
            nc.scalar.dma_start(abias[:], attn_b_d[:])
            cbias = sp.tile([128, 1], F32, tag="cbias")
            nc.scalar.dma_start(cbias[:], ctx_b_d[:])
            h0c = sp.tile([128, 1], BF16, tag="h0c")
            nc.scalar.dma_start(h0c[:], h0c_d[:])
            h0cm = sp.tile([128, 8], F32, tag="h0cm")
            nc.scalar.dma_start(h0cm[:], h0cm_d[:])
            bias8 = sp.tile([128, 48], F32, tag="bias8")
            nc.scalar.dma_start(bias8[:], bias8_d[:])
            obias = sp.tile([128, VT], F32, tag="obias")
            nc.scalar.dma_start(obias[:], out_b_d[:])

            attn_sb = wp.tile([128, 16 * LC], E3, tag="attn")
            nc.sync.dma_start(attn_sb[:, :8 * LC], attn_wt_d[:, :8 * LC])
            nc.sync.dma_start(attn_sb[:, 8 * LC:], attn_wt_d[:, 8 * LC:])
            enc_hi = wp.tile([128, 4 * H], BF16, tag="enchi")
            nc.sync.dma_start(enc_hi[:], enc_hi_d[:])
            enc_lo = wp.tile([128, 4 * H], E3, tag="enclo")
            nc.sync.dma_start(enc_lo[:], enc_lo_d[:])
            ctx_hi = wp.tile([128, 2048], BF16, tag="ctxhi")
            nc.sync.dma_start(ctx_hi[:], ctx_hi_d[:])
            ctx_lo = wp.tile([128, 1024], E3, tag="ctxlo")
            nc.sync.dma_start(ctx_lo[:], ctx_lo_d[:])
            wih_hi = wp.tile([128, 3 * H], BF16, tag="wihhi")
            nc.sync.dma_start(wih_hi[:], wih_hi_d[:])
            wih_lo = wp.tile([128, 3 * H], E3, tag="wihlo")
            nc.sync.dma_start(wih_lo[:], wih_lo_d[:])
            whh_sb = wp.tile([128, 3 * H], E3, tag="whh")
            nc.sync.dma_start(whh_sb[:], whh_d[:])

            # out_W tiles are allocated here but their DMAs are issued on
            # the gpsimd queue AFTER the CC1 trigger (below): the 6.5MB/core
            # stream would otherwise compete with every core's attention
            # prefix and scatter the collective entry times.
            outw = [wp.tile([128, VPAD], E3, tag=f"outw{j}", name=f"outw{j}")
                    for j in range(8)]

            # ------------- constants (DVE memsets, no DMA deps) -------------
            junkw = sp.tile([128, 8], BF16, tag="junkw")
            nc.vector.memset(junkw[:], 0.0)
            ones = sp.tile([128, 128], F32, tag="ones")
            nc.vector.memset(ones[:], 1.0)
            # rhs5 per l-tile t: cols 5t..5t+4 = [a_hi, a_lo, 1, a_hi/S, 1/S]
            rhs5 = sp.tile([128, 20], BF16, tag="rhs5")
            nc.vector.memset(rhs5[:, 2:20:5], 1.0)
            nc.vector.memset(rhs5[:, 4:20:5], 1.0 / S_ENC)
            pack_sb = sp.tile([128, 18], F32, tag="pack")
            nc.vector.memset(pack_sb[:, 17:18], 0.0)

            # ---------------- startup barrier ----------------
            # The first collective of a NEFF pays a 15-30us ncfw/channel
            # warmup before ranks get serviced.  Fire a dependency-free
            # dummy AllGather at t=0 so that cost overlaps the DMA prefix
            # instead of inflating CC1.
            bar_in = dp.tile([128, 1], F32, tag="barin")
            bar_out = dp.tile([NCORES, 128, 1], F32, tag="barout",
                              addr_space="Shared")
            nc.gpsimd.collective_compute(AG, BYP, replica_groups=RG,
                                         ins=[bar_in.opt()],
                                         outs=[bar_out.opt()])

            # ---------------- PE warmup (HAM clock-gate) ----------------
            # batch 1 runs from ~1us (memset-gated); batch 2 is gated on the
            # catin DMA (~6us) so activity bridges to the attention matmuls.
            junk_ps = pp.tile([8, 1], F32, tag="junkps")
            for _ in range(64):
                nc.tensor.matmul(junk_ps[:], junkw[:], junkw[:, 0:1],
                                 start=True, stop=True)
            for _ in range(48):
                nc.tensor.matmul(junk_ps[:], catin[:, 0:8], junkw[:, 0:1],
                                 start=True, stop=True)

            # ---------------- attention logits ----------------
            # a[l] for the 512 local l, laid out [128, 4] col-major tiles.
            # NOTE: accumulation groups must be contiguous in program order.
            a_psA = pp.tile([128, 4], F32, tag="apsA")
            for j in range(4):           # l tiles
                for i in range(8):       # xe half of the cat dim
                    nc.tensor.matmul(
                        a_psA[:, j:j + 1],
                        attn_sb[:, LC * i + 128 * j:LC * i + 128 * (j + 1)],
                        catins[:, i:i + 1],
                        start=(i == 0), stop=(i == 7))
            a_psB = pp.tile([128, 4], F32, tag="apsB")
            for j in range(4):           # l tiles
                for i in range(8, 16):   # h0 half
                    nc.tensor.matmul(
                        a_psB[:, j:j + 1],
                        attn_sb[:, LC * i + 128 * j:LC * i + 128 * (j + 1)],
                        catins[:, i:i + 1],
                        start=(i == 8), stop=(i == 15))

            # gh = W_hh @ h0 and the xe half of the ctx projection depend
            # only on inputs — run them while the DVE builds rhs5, and
            # before the CC1 wait.
            gih_ps = pp.tile([128, 48], F32, tag="gihps")
            for c in range(24):
                nc.tensor.matmul(gih_ps[:, 24 + c:25 + c],
                                 whh_sb[:, 128 * c:128 * (c + 1)], h0c[:],
                                 start=True, stop=True)
            g_ps = pp.tile([128, 2], F32, tag="gps")
            for i in range(8):
                nc.tensor.matmul(g_ps[:, 0:1], ctx_hi[:, 128 * i:128 * (i + 1)],
                                 catin[:, i:i + 1],
                                 start=(i == 0), stop=(i == 7))

            # a = psA + psB + bias; exp + row-sum fused via accum_out
            a_half = sp.tile([128, 4], F32, tag="ahalf")
            nc.vector.tensor_add(a_half[:], a_psA[:], abias[:])
            a_sb = sp.tile([128, 4], F32, tag="a")
            nc.vector.tensor_add(a_sb[:], a_psB[:], a_half[:])
            exp_sb = sp.tile([128, 4], F32, tag="expa")
            exp_r = sp.tile([128, 1], F32, tag="expr")
            nc.scalar.activation(exp_sb[:], a_sb[:], ACTF.Exp,
                                 accum_out=exp_r[:])
            # split a into hi/lo bf16 directly into the rhs5 columns
            nc.vector.tensor_copy(rhs5[:, 0:20:5], a_sb[:])
            a_hif = sp.tile([128, 4], F32, tag="ahif")
            nc.vector.tensor_copy(a_hif[:], rhs5[:, 0:20:5])
            nc.vector.tensor_sub(rhs5[:, 1:20:5], a_sb[:], a_hif[:])
            nc.vector.tensor_scalar_mul(rhs5[:, 3:20:5], rhs5[:, 0:20:5],
                                        1.0 / S_ENC)

            # pack psum cols per h-chunk c: 5c+0 ehi*ahi, +1 ehi*alo,
            # +2 ehi*1, +3 elo*ahi/S, +4 elo*1/S; col 40 = sum(exp a)
            # broadcast to all partitions via the ones matmul.
            pack_ps = pp.tile([128, 41], F32, tag="packps")
            for c in range(8):
                for j in range(4):
                    nc.tensor.matmul(
                        pack_ps[:, 5 * c:5 * c + 3],
                        enc_hi[:, H * j + 128 * c:H * j + 128 * (c + 1)],
                        rhs5[:, 5 * j:5 * j + 3],
                        start=(j == 0), stop=(j == 3))
                for j in range(4):
                    nc.tensor.matmul(
                        pack_ps[:, 5 * c + 3:5 * c + 5],
                        enc_lo[:, H * j + 128 * c:H * j + 128 * (c + 1)],
                        rhs5[:, 5 * j + 3:5 * j + 5],
                        start=(j == 0), stop=(j == 3))
            nc.tensor.matmul(pack_ps[:, 40:41], ones[:], exp_r[:],
                             start=True, stop=True)

            # combine: pack[0:8]=P, pack[8:16]=S, pack[16]=s, pack[17]=0
            # (ops can read at most one PSUM input -> copy to SBUF first)
            pk_sb = sp.tile([128, 41], F32, tag="pksb")
            nc.vector.tensor_copy(pk_sb[:], pack_ps[:])
            nc.vector.tensor_add(pack_sb[:, 0:8], pk_sb[:, 0:40:5],
                                 pk_sb[:, 1:40:5])
            nc.vector.tensor_add(pack_sb[:, 0:8], pack_sb[:, 0:8],
                                 pk_sb[:, 3:40:5])
            nc.vector.tensor_add(pack_sb[:, 8:16], pk_sb[:, 2:40:5],
                                 pk_sb[:, 4:40:5])
            nc.vector.tensor_copy(pack_sb[:, 16:17], pk_sb[:, 40:41])
            # stash the xe half of ctx while the PE is free
            g_xe = sp.tile([128, 1], F32, tag="gxe")
            nc.vector.tensor_copy(g_xe[:], g_ps[:, 0:1])

            # ---------------- collective 1: AllGather + fold ----------------
            # all collective-adjacent DMAs ride the gpsimd SWDGE lanes so
            # their (late) completions never block the HWDGE weight-stream
            # semaphore lanes.
            cc1_in = dp.tile([128, 18], F32, tag="cc1in")
            cc1_out = dp.tile([NCORES, 128, 18], F32, tag="cc1out",
                              addr_space="Shared")
            nc.scalar.dma_start(cc1_in[:], pack_sb[:])
            nc.gpsimd.collective_compute(AG, BYP, replica_groups=RG,
                                         ins=[cc1_in.opt()],
                                         outs=[cc1_out.opt()])
            # Gate the big out_W stream behind the CC1 input: a dummy DVE
            # write into each outw tile that READS pack_sb forces the DMA
            # (write-after-write) to wait until the attention prefix is
            # done, so it never competes with any core's collective entry.
            for j in range(8):
                nc.vector.tensor_copy(outw[j][:, 0:4].bitcast(F32),
                                      pack_sb[:, 0:1])
            for j in range(8):
                nc.sync.dma_start(outw[j][:],
                                  out_wt_d[128 * j:128 * (j + 1), :])
            sums3 = sp.tile([128, NCORES, 18], F32, tag="sums3")
            nc.scalar.dma_start(sums3[:], cc1_out[:].transpose([1, 0, 2]))
            nc.vector.tensor_add(sums3[:, 0:4, :], sums3[:, 0:4, :],
                                 sums3[:, 4:8, :])
            nc.vector.tensor_add(sums3[:, 0:2, :], sums3[:, 0:2, :],
                                 sums3[:, 2:4, :])
            nc.vector.tensor_add(sums3[:, 0:1, :], sums3[:, 0:1, :],
                                 sums3[:, 1:2, :])
            sums = sums3[:, 0, :]

            # PE keep-warm during the CC1 wait
            for _ in range(300):
                nc.tensor.matmul(a_psA[:8, 0:1], junkw[:], junkw[:, 0:1],
                                 start=True, stop=True)

            # c = log(sum exp) — s was broadcast to all partitions pre-CC
            c_sb = sp.tile([128, 1], F32, tag="c")
            nc.scalar.activation(c_sb[:], sums[:, 16:17], ACTF.Ln)
            # attn_applied = P - c * S   [128, 8]
            atmp = sp.tile([128, 8], F32, tag="atmp")
            nc.vector.tensor_scalar(atmp[:], sums[:, 8:16], c_sb[:], None,
                                    mybir.AluOpType.mult)
            attnap = sp.tile([128, 8], F32, tag="attnap")
            nc.vector.tensor_sub(attnap[:], sums[:, 0:8], atmp[:])
            # ctx rhs cols per m: [ap_hi, ap_lo, ap_hi/S_CTX]
            aprhs = sp.tile([128, 24], BF16, tag="aprhs")
            nc.vector.tensor_copy(aprhs[:, 0:24:3], attnap[:])
            ap_hif = sp.tile([128, 8], F32, tag="aphif")
            nc.vector.tensor_copy(ap_hif[:], aprhs[:, 0:24:3])
            nc.vector.tensor_sub(aprhs[:, 1:24:3], attnap[:], ap_hif[:])
            nc.vector.tensor_scalar_mul(aprhs[:, 2:24:3], aprhs[:, 0:24:3],
                                        1.0 / S_CTX)

            # ---------------- context projection (attnap half) ----------
            nmm = 24
            k = 0
            for m in range(8):
                t = ctx_hi[:, 128 * (8 + m):128 * (9 + m)]
                nc.tensor.matmul(g_ps[:, 1:2], t, aprhs[:, 3 * m:3 * m + 1],
                                 start=(k == 0), stop=(k == nmm - 1))
                k += 1
                nc.tensor.matmul(g_ps[:, 1:2], t, aprhs[:, 3 * m + 1:3 * m + 2],
                                 start=False, stop=(k == nmm - 1))
                k += 1
                nc.tensor.matmul(g_ps[:, 1:2], ctx_lo[:, 128 * m:128 * (m + 1)],
                                 aprhs[:, 3 * m + 2:3 * m + 3],
                                 start=False, stop=(k == nmm - 1))
                k += 1
            gpre = sp.tile([128, 1], F32, tag="gpre")
            nc.vector.tensor_add(gpre[:], g_ps[:, 1:2], g_xe[:])
            g_sb = sp.tile([128, 1], F32, tag="g")
            nc.scalar.activation(g_sb[:], gpre[:], ACTF.Relu, bias=cbias[:])
            # GRU rhs cols: [g_hi, g_lo, g_hi/S_WIH]
            grhs = sp.tile([128, 3], BF16, tag="grhs")
            nc.vector.tensor_copy(grhs[:, 0:1], g_sb[:])
            g_hif = sp.tile([128, 1], F32, tag="ghif")
            nc.vector.tensor_copy(g_hif[:], grhs[:, 0:1])
            nc.vector.tensor_sub(grhs[:, 1:2], g_sb[:], g_hif[:])
            nc.vector.tensor_scalar_mul(grhs[:, 2:3], grhs[:, 0:1],
                                        1.0 / S_WIH)

            # ---------------- GRU partial matvecs (gi; gh ran earlier) ----
            for c in range(24):
                whi = wih_hi[:, 128 * c:128 * (c + 1)]
                nc.tensor.matmul(gih_ps[:, c:c + 1], whi, grhs[:, 0:1],
                                 start=True, stop=False)
                nc.tensor.matmul(gih_ps[:, c:c + 1], whi, grhs[:, 1:2],
                                 start=False, stop=False)
                nc.tensor.matmul(gih_ps[:, c:c + 1],
                                 wih_lo[:, 128 * c:128 * (c + 1)],
                                 grhs[:, 2:3],
                                 start=False, stop=True)
            # fold biases/8 in here: the 8-way fold sum restores them
            pack2 = sp.tile([128, 48], F32, tag="pack2")
            nc.vector.tensor_add(pack2[:], gih_ps[:], bias8[:])

            # ---------------- collective 2: AllGather + fold ----------------
            cc2_in = dp.tile([128, 48], F32, tag="cc2in")
            cc2_out = dp.tile([NCORES, 128, 48], F32, tag="cc2out",
                              addr_space="Shared")
            nc.scalar.dma_start(cc2_in[:], pack2[:])
            nc.gpsimd.collective_compute(AG, BYP, replica_groups=RG,
                                         ins=[cc2_in.opt()],
                                         outs=[cc2_out.opt()])
            gsum3 = sp.tile([128, NCORES, 48], F32, tag="gsum3")
            nc.scalar.dma_start(gsum3[:], cc2_out[:].transpose([1, 0, 2]))
            nc.vector.tensor_add(gsum3[:, 0:4, :], gsum3[:, 0:4, :],
                                 gsum3[:, 4:8, :])
            nc.vector.tensor_add(gsum3[:, 0:2, :], gsum3[:, 0:2, :],
                                 gsum3[:, 2:4, :])
            nc.vector.tensor_add(gsum3[:, 0:1, :], gsum3[:, 0:1, :],
                                 gsum3[:, 1:2, :])
            gsum = gsum3[:, 0, :]

            # PE keep-warm during the CC2 wait, then re-warm gated on the
            # CC2 results so the out-projection starts at full clock
            for _ in range(140):
                nc.tensor.matmul(a_psB[:8, 0:1], junkw[:], junkw[:, 0:1],
                                 start=True, stop=True)

            for _ in range(120):
                nc.tensor.matmul(a_psB[:8, 0:1], gsum3[:, 0, 0:8],
                                 ones[:, 0:1], start=True, stop=True)

            # gates (PyTorch order r, z, n); r and z share one Sigmoid pass
            rzpre = sp.tile([128, 16], F32, tag="rzpre")
            nc.vector.tensor_add(rzpre[:], gsum[:, 0:16], gsum[:, 24:40])
            rz_sb = sp.tile([128, 16], F32, tag="rz")
            nc.scalar.activation(rz_sb[:], rzpre[:], ACTF.Sigmoid)
            npre = sp.tile([128, 8], F32, tag="npre")
            nc.vector.tensor_mul(npre[:], rz_sb[:, 0:8], gsum[:, 40:48])
            nc.vector.tensor_add(npre[:], npre[:], gsum[:, 16:24])
            n_sb = sp.tile([128, 8], F32, tag="n")
            nc.scalar.activation(n_sb[:], npre[:], ACTF.Tanh)
            # h_new = n + z * (h0 - n), then scale by 1/S_OUT for the
            # e3m4 out projection (exact power-of-2 shift in bf16)
            hd = sp.tile([128, 8], F32, tag="hd")
            nc.vector.tensor_sub(hd[:], h0cm[:], n_sb[:])
            nc.vector.tensor_mul(hd[:], hd[:], rz_sb[:, 8:16])
            hsum = sp.tile([128, 8], F32, tag="hsum")
            nc.vector.tensor_add(hsum[:], n_sb[:], hd[:])
            hnew_b = sp.tile([128, 8], BF16, tag="hnewb")
            nc.vector.tensor_scalar_mul(hnew_b[:], hsum[:], 1.0 / S_OUT)

            # ---------------- output projection ----------------
            # h-chunk-major so phase j only needs outw[j] (DMA-paced start);
            # DVE accumulates phases into logit_sb.
            logit_sb = sp.tile([128, VT], F32, tag="logit")
            for j in range(8):
                o_ps = pp.tile([128, VT], F32, tag="ops", bufs=2)
                for t in range(VT):
                    nc.tensor.matmul(
                        o_ps[:, t:t + 1],
                        outw[j][:, 128 * t:128 * (t + 1)],
                        hnew_b[:, j:j + 1],
                        start=True, stop=True)
                if j == 0:
                    nc.vector.tensor_add(logit_sb[:], o_ps[:], obias[:])
                else:
                    nc.vector.tensor_add(logit_sb[:], logit_sb[:], o_ps[:])

            nc.gpsimd.dma_start(out_d[:], logit_sb[:])

            if dbg:
                dbg_tiles = {
                    "dbg_a": a_sb, "dbg_exp": exp_sb, "dbg_pack": pack_sb,
                    "dbg_sums": sums, "dbg_attnap": attnap,
                    "dbg_g": g_sb, "dbg_pack2": pack2, "dbg_gsum": gsum,
                    "dbg_hnew": hnew_b,
                }
                for name, t in dbg_tiles.items():
                    shp = list(t[:].shape)
                    d = nc.dram_tensor(name, shp, t[:].dtype,
                                       kind="ExternalOutput")
                    nc.sync.dma_start(d[:], t[:])

    nc.compile()
    _CACHE[key] = nc
    return nc


def _col_major(v, ncols):
    # v [n] -> [128, ncols] with [p, c] = v[128 * c + p]
    return np.ascontiguousarray(v.reshape(ncols, 128).T)


def _pack_rows(a, nb):
    # a [nb*128, w] -> [128, nb*w] with [p, w*i + q] = a[128*i + p, q]
    w = a.shape[1]
    return np.ascontiguousarray(
        a.reshape(nb, 128, w).transpose(1, 0, 2).reshape(128, nb * w))


def _shard(inputs):
    x = np.asarray(inputs["x"]).reshape(-1)
    h0 = np.asarray(inputs["h"], dtype=np.float32).reshape(H)
    enc = np.asarray(inputs["encoder_outputs"], dtype=np.float32)
    emb = np.asarray(inputs["emb"])
    attn_W = np.asarray(inputs["attn_W"], dtype=np.float32)
    attn_b = np.asarray(inputs["attn_b"], dtype=np.float32)
    ctx_W = np.asarray(inputs["ctx_W"], dtype=np.float32)
    ctx_b = np.asarray(inputs["ctx_b"], dtype=np.float32)
    W_ih = np.asarray(inputs["W_ih"], dtype=np.float32)
    W_hh = np.asarray(inputs["W_hh"], dtype=np.float32)
    b_ih = np.asarray(inputs["b_ih"], dtype=np.float32)
    b_hh = np.asarray(inputs["b_hh"], dtype=np.float32)
    out_W = np.asarray(inputs["out_W"], dtype=np.float32)
    out_b = np.asarray(inputs["out_b"], dtype=np.float32)

    xe = np.asarray(emb[int(x[0])], dtype=np.float32)
    catin = np.concatenate([_col_major(xe, 8), _col_major(h0, 8)],
                           axis=1).astype(NPBF16)
    h0cm = _col_major(h0, 8)

    bias8 = np.concatenate([_col_major(b_ih, 24), _col_major(b_hh, 24)],
                           axis=1) / 8.0

    in_maps = []
    for k in range(NCORES):
        lsl = slice(LC * k, LC * (k + 1))
        hsl = slice(HC * k, HC * (k + 1))
        v0, v1 = VC * k, min(VC * (k + 1), V)
        owt = np.zeros((H, VPAD), dtype=NPE3)
        owt[:, :v1 - v0] = (out_W[v0:v1, :].T * S_OUT).astype(NPE3)
        ob = np.zeros(VPAD, dtype=np.float32)
        ob[:v1 - v0] = out_b[v0:v1]

        enc_k = enc[lsl, :]
        enc_hi = enc_k.astype(NPBF16)
        enc_lo = ((enc_k - enc_hi.astype(np.float32)) * S_ENC).astype(NPE3)
        ctxT = np.ascontiguousarray(ctx_W[hsl, :].T)       # [2048, 128]
        ctx_hi = ctxT.astype(NPBF16)
        ctx_lo = ((ctxT[H:] - ctx_hi[H:].astype(np.float32))
                  * S_CTX).astype(NPE3)
        wihT = np.ascontiguousarray(W_ih[:, hsl].T)        # [128, 3072]
        wih_hi = wihT.astype(NPBF16)
        wih_lo = ((wihT - wih_hi.astype(np.float32)) * S_WIH).astype(NPE3)
        whhT = np.ascontiguousarray(W_hh[:, hsl].T)
        whh = (whhT * S_WHH).astype(NPE3)

        in_maps.append({
            "catin": np.ascontiguousarray(catin),
            "catins": np.ascontiguousarray(
                (catin.astype(np.float32) / S_ATTN).astype(NPBF16)),
            "attn_wt": _pack_rows((attn_W[lsl, :].T * S_ATTN).astype(NPE3),
                                  16),
            "attn_b": _col_major(attn_b[lsl], 4),
            "enc_hi": _pack_rows(enc_hi, 4),
            "enc_lo": _pack_rows(enc_lo, 4),
            "ctx_hi": _pack_rows(ctx_hi, 16),
            "ctx_lo": _pack_rows(ctx_lo, 8),
            "ctx_b": ctx_b[hsl].reshape(128, 1).copy(),
            "wih_hi": wih_hi,
            "wih_lo": wih_lo,
            "whh": whh,
            "h0c": (h0[hsl] / S_WHH).reshape(128, 1).astype(NPBF16),
            "h0cm": h0cm,
            "bias8": np.ascontiguousarray(bias8, dtype=np.float32),
            "out_wt": owt,
            "out_b": _col_major(ob, VT),
        })
    return in_maps


def _gather(results):
    logits = np.empty(NCORES * VC, dtype=np.float32)
    for k in range(NCORES):
        chunk = np.asarray(results[k]["out"]).T.ravel()   # [VT*128]
        logits[VC * k:VC * (k + 1)] = chunk[:VC]
    return logits[:V].reshape(1, V)


def kernel(**inputs):
    nc = _build()
    in_maps = _shard(inputs)
    try:
        res = run_bass_kernel_spmd(nc, in_maps, core_ids=list(range(NCORES)))
    except Exception:
        # A dirty device state from a previous process occasionally fails
        # the first launch (NRT_EXEC_UNIT_UNRECOVERABLE); one retry clears.
        res = run_bass_kernel_spmd(nc, in_maps, core_ids=list(range(NCORES)))
    return _gather(res.results)


def kernel_traced(**inputs):
    """Like kernel() but profiles on HW; returns (output, exec_time_ns)."""
    nc = _build()
    in_maps = _shard(inputs)
    res = run_bass_kernel_spmd(nc, in_maps, core_ids=list(range(NCORES)),
                               trace=True)
    return _gather(res.results), res.exec_time_ns


def kernel_debug(**inputs):
    """Run the debug build; returns per-core dicts of all outputs."""
    nc = _build(dbg=True)
    in_maps = _shard(inputs)
    res = run_bass_kernel_spmd(nc, in_maps, core_ids=list(range(NCORES)))
    return res.results


# revision 14
# speedup vs baseline: 1.0085x; 1.0085x over previous
"""Trainium2 Bass kernel for a single-step GRU attention decoder.

Math (matches the reference nn.Module):
    xe  = emb[x]                                   # [H]
    a   = log_softmax(cat(xe, h0) @ attn_W.T + attn_b)   # [L]
    ap  = a @ encoder_outputs                      # [H]
    g   = relu(cat(xe, ap) @ ctx_W.T + ctx_b)      # [H]
    GRU(g, h0) -> h_new                            # [H]
    logits = h_new @ out_W.T + out_b               # [V]

Distribution across 8 NeuronCores (one TRN2 chip):
  - attention sharded over L (512 rows/core); exploiting linearity,
    log_softmax @ enc == a @ enc - (log sum exp a) * colsum(enc), so one
    collective of per-core partials {a@enc, colsum(enc), sum(exp a)} lets
    every core reconstruct attn_applied locally.
  - ctx projection sharded by output rows; GRU mats sharded by *input*
    columns so a single collective of partial (gi, gh) lets every core
    compute the full gates / h_new locally.
  - out projection sharded over vocab (6283 rows/core, padded to 6400).

Both collectives are AllGathers (≈4.6us floor vs ≈10us for AllReduce on
8 cores) followed by a 3-level on-chip fold (log2(8) tensor_adds).

Precision: bulk weights are bf16; "lo" residual terms (enc, ctx ap-part,
W_ih) and the *whole* of W_hh / out_W are fp8 e3m4 with power-of-2
scales folded into the matmul rhs (exact in bf16), halving out_W HBM
traffic.  fp32 accumulation in PSUM throughout; gates/softmax in fp32.
End-to-end rel err ~1.2e-2 (sim) vs the 2e-2 gate.

The PE idles during collective waits; dummy matmuls are issued in those
windows to keep the HAM clock-gate warm (PE at 2.4GHz, not 1.2GHz).
"""

import ml_dtypes
import numpy as np

import concourse.bass as bass
import concourse.bacc as bacc
import concourse.tile as tile
from concourse import mybir
from concourse.bass_utils import run_bass_kernel_spmd

H = 1024
V = 50257
L = 4096
NCORES = 8
LC = L // NCORES          # 512 encoder rows per core
HC = H // NCORES          # 128 hidden chunk per core
VC = -(-V // NCORES)      # 6283 vocab rows per core
VT = 50                   # vocab tiles of 128 per core
VPAD = VT * 128           # 6400
F32 = mybir.dt.float32
BF16 = mybir.dt.bfloat16
E3 = mybir.dt.float8e3
NPBF16 = ml_dtypes.bfloat16
NPE3 = ml_dtypes.float8_e3m4
RG = [list(range(NCORES))]

# power-of-2 scales for the e3m4 tensors (folded into rhs; exact in bf16)
S_ENC = 512.0
S_CTX = 2048.0
S_WIH = 2048.0
S_WHH = 256.0
S_OUT = 256.0
S_ATTN = 256.0

_CACHE = {}


def _build(dbg=False):
    key = ("nc", dbg)
    if key in _CACHE:
        return _CACHE[key]

    nc = bacc.Bacc("TRN2", target_bir_lowering=False, debug=False,
                   num_devices=NCORES)

    def din(name, shape, dt=F32):
        return nc.dram_tensor(name, shape, dt, kind="ExternalInput")

    catin_d = din("catin", [128, 16], BF16)    # cols 0-7 xe, 8-15 h0
    catins_d = din("catins", [128, 16], BF16)  # catin / S_ATTN (for e3 attn)
    attn_wt_d = din("attn_wt", [128, 16 * LC], E3)
    attn_b_d = din("attn_b", [128, 4])
    enc_hi_d = din("enc_hi", [128, 4 * H], BF16)
    enc_lo_d = din("enc_lo", [128, 4 * H], E3)
    ctx_hi_d = din("ctx_hi", [128, 2048], BF16)
    ctx_lo_d = din("ctx_lo", [128, 1024], E3)
    ctx_b_d = din("ctx_b", [128, 1])
    wih_hi_d = din("wih_hi", [HC, 3 * H], BF16)
    wih_lo_d = din("wih_lo", [HC, 3 * H], E3)
    whh_d = din("whh", [HC, 3 * H], E3)
    h0c_d = din("h0c", [128, 1], BF16)         # h0 chunk k / S_WHH
    h0cm_d = din("h0cm", [128, 8])             # full h0, col-major, fp32
    bias8_d = din("bias8", [128, 48])          # cat(b_ih, b_hh)/8 col-major
    out_wt_d = din("out_wt", [H, VPAD], E3)    # out_W vocab chunk, transposed
    out_b_d = din("out_b", [128, VT])
    out_d = nc.dram_tensor("out", [128, VT], F32, kind="ExternalOutput")

    AG = "AllGather"
    BYP = mybir.AluOpType.bypass
    ACTF = mybir.ActivationFunctionType

    with tile.TileContext(nc) as tc:
        with (
            tc.tile_pool(name="wp", bufs=1) as wp,
            tc.tile_pool(name="sp", bufs=1) as sp,
            tc.tile_pool(name="pp", bufs=1, space="PSUM") as pp,
            tc.tile_pool(name="dp", bufs=1, space="DRAM") as dp,
        ):
            # ------------- loads (issue order = priority order) -------------
            # small tiles go on the scalar HWDGE ring (qAct) so the sync
            # ring (qSP) starts streaming the big prefix immediately; both
            # rings' DMAs complete early so no sem-lane entanglement.
            catin = sp.tile([128, 16], BF16, tag="catin")
            nc.scalar.dma_start(catin[:], catin_d[:])
            catins = sp.tile([128, 16], BF16, tag="catins")
            nc.scalar.dma_start(catins[:], catins_d[:])
            abias = sp.tile([128, 4], F32, tag="abias")
            nc.scalar.dma_start(abias[:], attn_b_d[:])
            cbias = sp.tile([128, 1], F32, tag="cbias")
            nc.scalar.dma_start(cbias[:], ctx_b_d[:])
            h0c = sp.tile([128, 1], BF16, tag="h0c")
            nc.scalar.dma_start(h0c[:], h0c_d[:])
            h0cm = sp.tile([128, 8], F32, tag="h0cm")
            nc.scalar.dma_start(h0cm[:], h0cm_d[:])
            bias8 = sp.tile([128, 48], F32, tag="bias8")
            nc.scalar.dma_start(bias8[:], bias8_d[:])
            obias = sp.tile([128, VT], F32, tag="obias")
            nc.scalar.dma_start(obias[:], out_b_d[:])

            attn_sb = wp.tile([128, 16 * LC], E3, tag="attn")
            nc.sync.dma_start(attn_sb[:, :8 * LC], attn_wt_d[:, :8 * LC])
            nc.sync.dma_start(attn_sb[:, 8 * LC:], attn_wt_d[:, 8 * LC:])
            enc_hi = wp.tile([128, 4 * H], BF16, tag="enchi")
            nc.sync.dma_start(enc_hi[:], enc_hi_d[:])
            enc_lo = wp.tile([128, 4 * H], E3, tag="enclo")
            nc.sync.dma_start(enc_lo[:], enc_lo_d[:])
            ctx_hi = wp.tile([128, 2048], BF16, tag="ctxhi")
            nc.sync.dma_start(ctx_hi[:], ctx_hi_d[:])
            ctx_lo = wp.tile([128, 1024], E3, tag="ctxlo")
            nc.sync.dma_start(ctx_lo[:], ctx_lo_d[:])
            wih_hi = wp.tile([128, 3 * H], BF16, tag="wihhi")
            nc.sync.dma_start(wih_hi[:], wih_hi_d[:])
            wih_lo = wp.tile([128, 3 * H], E3, tag="wihlo")
            nc.sync.dma_start(wih_lo[:], wih_lo_d[:])
            whh_sb = wp.tile([128, 3 * H], E3, tag="whh")
            nc.sync.dma_start(whh_sb[:], whh_d[:])

            # out_W tiles are allocated here but their DMAs are issued on
            # the gpsimd queue AFTER the CC1 trigger (below): the 6.5MB/core
            # stream would otherwise compete with every core's attention
            # prefix and scatter the collective entry times.
            outw = [wp.tile([128, VPAD], E3, tag=f"outw{j}", name=f"outw{j}")
                    for j in range(8)]

            # ------------- constants (DVE memsets, no DMA deps) -------------
            junkw = sp.tile([128, 8], BF16, tag="junkw")
            nc.vector.memset(junkw[:], 0.0)
            ones = sp.tile([128, 128], F32, tag="ones")
            nc.vector.memset(ones[:], 1.0)
            # rhs5 per l-tile t: cols 5t..5t+4 = [a_hi, a_lo, 1, a_hi/S, 1/S]
            rhs5 = sp.tile([128, 20], BF16, tag="rhs5")
            nc.vector.memset(rhs5[:, 2:20:5], 1.0)
            nc.vector.memset(rhs5[:, 4:20:5], 1.0 / S_ENC)
            pack_sb = sp.tile([128, 18], F32, tag="pack")
            nc.vector.memset(pack_sb[:, 17:18], 0.0)

            # ---------------- PE warmup (HAM clock-gate) ----------------
            # batch 1 runs from ~1us (memset-gated); batch 2 is gated on the
            # catin DMA (~6us) so activity bridges to the attention matmuls.
            junk_ps = pp.tile([8, 1], F32, tag="junkps")
            for _ in range(64):
                nc.tensor.matmul(junk_ps[:], junkw[:], junkw[:, 0:1],
                                 start=True, stop=True)
            for _ in range(48):
                nc.tensor.matmul(junk_ps[:], catin[:, 0:8], junkw[:, 0:1],
                                 start=True, stop=True)

            # ---------------- attention logits ----------------
            # a[l] for the 512 local l, laid out [128, 4] col-major tiles.
            # NOTE: accumulation groups must be contiguous in program order.
            a_psA = pp.tile([128, 4], F32, tag="apsA")
            for j in range(4):           # l tiles
                for i in range(8):       # xe half of the cat dim
                    nc.tensor.matmul(
                        a_psA[:, j:j + 1],
                        attn_sb[:, LC * i + 128 * j:LC * i + 128 * (j + 1)],
                        catins[:, i:i + 1],
                        start=(i == 0), stop=(i == 7))
            a_psB = pp.tile([128, 4], F32, tag="apsB")
            for j in range(4):           # l tiles
                for i in range(8, 16):   # h0 half
                    nc.tensor.matmul(
                        a_psB[:, j:j + 1],
                        attn_sb[:, LC * i + 128 * j:LC * i + 128 * (j + 1)],
                        catins[:, i:i + 1],
                        start=(i == 8), stop=(i == 15))

            # gh = W_hh @ h0 and the xe half of the ctx projection depend
            # only on inputs — run them while the DVE builds rhs5, and
            # before the CC1 wait.
            gih_ps = pp.tile([128, 48], F32, tag="gihps")
            for c in range(24):
                nc.tensor.matmul(gih_ps[:, 24 + c:25 + c],
                                 whh_sb[:, 128 * c:128 * (c + 1)], h0c[:],
                                 start=True, stop=True)
            g_ps = pp.tile([128, 2], F32, tag="gps")
            for i in range(8):
                nc.tensor.matmul(g_ps[:, 0:1], ctx_hi[:, 128 * i:128 * (i + 1)],
                                 catin[:, i:i + 1],
                                 start=(i == 0), stop=(i == 7))

            # a = psA + psB + bias; exp + row-sum fused via accum_out
            a_half = sp.tile([128, 4], F32, tag="ahalf")
            nc.vector.tensor_add(a_half[:], a_psA[:], abias[:])
            a_sb = sp.tile([128, 4], F32, tag="a")
            nc.vector.tensor_add(a_sb[:], a_psB[:], a_half[:])
            exp_sb = sp.tile([128, 4], F32, tag="expa")
            exp_r = sp.tile([128, 1], F32, tag="expr")
            nc.scalar.activation(exp_sb[:], a_sb[:], ACTF.Exp,
                                 accum_out=exp_r[:])
            # split a into hi/lo bf16 directly into the rhs5 columns
            nc.vector.tensor_copy(rhs5[:, 0:20:5], a_sb[:])
            a_hif = sp.tile([128, 4], F32, tag="ahif")
            nc.vector.tensor_copy(a_hif[:], rhs5[:, 0:20:5])
            nc.vector.tensor_sub(rhs5[:, 1:20:5], a_sb[:], a_hif[:])
            nc.vector.tensor_scalar_mul(rhs5[:, 3:20:5], rhs5[:, 0:20:5],
                                        1.0 / S_ENC)

            # pack psum cols per h-chunk c: 5c+0 ehi*ahi, +1 ehi*alo,
            # +2 ehi*1, +3 elo*ahi/S, +4 elo*1/S; col 40 = sum(exp a)
            # broadcast to all partitions via the ones matmul.
            pack_ps = pp.tile([128, 41], F32, tag="packps")
            for c in range(8):
                for j in range(4):
                    nc.tensor.matmul(
                        pack_ps[:, 5 * c:5 * c + 3],
                        enc_hi[:, H * j + 128 * c:H * j + 128 * (c + 1)],
                        rhs5[:, 5 * j:5 * j + 3],
                        start=(j == 0), stop=(j == 3))
                for j in range(4):
                    nc.tensor.matmul(
                        pack_ps[:, 5 * c + 3:5 * c + 5],
                        enc_lo[:, H * j + 128 * c:H * j + 128 * (c + 1)],
                        rhs5[:, 5 * j + 3:5 * j + 5],
                        start=(j == 0), stop=(j == 3))
            nc.tensor.matmul(pack_ps[:, 40:41], ones[:], exp_r[:],
                             start=True, stop=True)

            # combine: pack[0:8]=P, pack[8:16]=S, pack[16]=s, pack[17]=0
            # (ops can read at most one PSUM input -> copy to SBUF first)
            pk_sb = sp.tile([128, 41], F32, tag="pksb")
            nc.vector.tensor_copy(pk_sb[:], pack_ps[:])
            nc.vector.tensor_add(pack_sb[:, 0:8], pk_sb[:, 0:40:5],
                                 pk_sb[:, 1:40:5])
            nc.vector.tensor_add(pack_sb[:, 0:8], pack_sb[:, 0:8],
                                 pk_sb[:, 3:40:5])
            nc.vector.tensor_add(pack_sb[:, 8:16], pk_sb[:, 2:40:5],
                                 pk_sb[:, 4:40:5])
            nc.vector.tensor_copy(pack_sb[:, 16:17], pk_sb[:, 40:41])
            # stash the xe half of ctx while the PE is free
            g_xe = sp.tile([128, 1], F32, tag="gxe")
            nc.vector.tensor_copy(g_xe[:], g_ps[:, 0:1])

            # ---------------- collective 1: AllGather + fold ----------------
            # all collective-adjacent DMAs ride the gpsimd SWDGE lanes so
            # their (late) completions never block the HWDGE weight-stream
            # semaphore lanes.
            cc1_in = dp.tile([128, 18], F32, tag="cc1in")
            cc1_out = dp.tile([NCORES, 128, 18], F32, tag="cc1out",
                              addr_space="Shared")
            nc.scalar.dma_start(cc1_in[:], pack_sb[:])
            nc.gpsimd.collective_compute(AG, BYP, replica_groups=RG,
                                         ins=[cc1_in.opt()],
                                         outs=[cc1_out.opt()])
            sums3 = sp.tile([128, NCORES, 18], F32, tag="sums3")
            nc.scalar.dma_start(sums3[:], cc1_out[:].transpose([1, 0, 2]))
            # The collective mesh makes no progress while bulk DMA is in
            # flight, so the 6.5MB/core out_W stream is released in two
            # waves into the collective-free windows: tiles 0-4 after the
            # CC1 result lands (streams during the local ctx/GRU chain),
            # tiles 5-7 after CC2 (streams during the gate math).  The
            # gate is a dummy DVE write into each tile reading a region
            # the folds don't touch, so it depends only on the return DMA.
            for j in range(5):
                nc.vector.tensor_copy(outw[j][:, 0:4].bitcast(F32),
                                      sums3[:, 7, 0:1])
            for j in range(5):
                nc.sync.dma_start(outw[j][:],
                                  out_wt_d[128 * j:128 * (j + 1), :])
            nc.vector.tensor_add(sums3[:, 0:4, :], sums3[:, 0:4, :],
                                 sums3[:, 4:8, :])
            nc.vector.tensor_add(sums3[:, 0:2, :], sums3[:, 0:2, :],
                                 sums3[:, 2:4, :])
            nc.vector.tensor_add(sums3[:, 0:1, :], sums3[:, 0:1, :],
                                 sums3[:, 1:2, :])
            sums = sums3[:, 0, :]

            # PE keep-warm during the CC1 wait
            for _ in range(300):
                nc.tensor.matmul(a_psA[:8, 0:1], junkw[:], junkw[:, 0:1],
                                 start=True, stop=True)

            # c = log(sum exp) — s was broadcast to all partitions pre-CC
            c_sb = sp.tile([128, 1], F32, tag="c")
            nc.scalar.activation(c_sb[:], sums[:, 16:17], ACTF.Ln)
            # attn_applied = P - c * S   [128, 8]
            atmp = sp.tile([128, 8], F32, tag="atmp")
            nc.vector.tensor_scalar(atmp[:], sums[:, 8:16], c_sb[:], None,
                                    mybir.AluOpType.mult)
            attnap = sp.tile([128, 8], F32, tag="attnap")
            nc.vector.tensor_sub(attnap[:], sums[:, 0:8], atmp[:])
            # ctx rhs cols per m: [ap_hi, ap_lo, ap_hi/S_CTX]
            aprhs = sp.tile([128, 24], BF16, tag="aprhs")
            nc.vector.tensor_copy(aprhs[:, 0:24:3], attnap[:])
            ap_hif = sp.tile([128, 8], F32, tag="aphif")
            nc.vector.tensor_copy(ap_hif[:], aprhs[:, 0:24:3])
            nc.vector.tensor_sub(aprhs[:, 1:24:3], attnap[:], ap_hif[:])
            nc.vector.tensor_scalar_mul(aprhs[:, 2:24:3], aprhs[:, 0:24:3],
                                        1.0 / S_CTX)

            # ---------------- context projection (attnap half) ----------
            nmm = 24
            k = 0
            for m in range(8):
                t = ctx_hi[:, 128 * (8 + m):128 * (9 + m)]
                nc.tensor.matmul(g_ps[:, 1:2], t, aprhs[:, 3 * m:3 * m + 1],
                                 start=(k == 0), stop=(k == nmm - 1))
                k += 1
                nc.tensor.matmul(g_ps[:, 1:2], t, aprhs[:, 3 * m + 1:3 * m + 2],
                                 start=False, stop=(k == nmm - 1))
                k += 1
                nc.tensor.matmul(g_ps[:, 1:2], ctx_lo[:, 128 * m:128 * (m + 1)],
                                 aprhs[:, 3 * m + 2:3 * m + 3],
                                 start=False, stop=(k == nmm - 1))
                k += 1
            gpre = sp.tile([128, 1], F32, tag="gpre")
            nc.vector.tensor_add(gpre[:], g_ps[:, 1:2], g_xe[:])
            g_sb = sp.tile([128, 1], F32, tag="g")
            nc.scalar.activation(g_sb[:], gpre[:], ACTF.Relu, bias=cbias[:])
            # GRU rhs cols: [g_hi, g_lo, g_hi/S_WIH]
            grhs = sp.tile([128, 3], BF16, tag="grhs")
            nc.vector.tensor_copy(grhs[:, 0:1], g_sb[:])
            g_hif = sp.tile([128, 1], F32, tag="ghif")
            nc.vector.tensor_copy(g_hif[:], grhs[:, 0:1])
            nc.vector.tensor_sub(grhs[:, 1:2], g_sb[:], g_hif[:])
            nc.vector.tensor_scalar_mul(grhs[:, 2:3], grhs[:, 0:1],
                                        1.0 / S_WIH)

            # ---------------- GRU partial matvecs (gi; gh ran earlier) ----
            for c in range(24):
                whi = wih_hi[:, 128 * c:128 * (c + 1)]
                nc.tensor.matmul(gih_ps[:, c:c + 1], whi, grhs[:, 0:1],
                                 start=True, stop=False)
                nc.tensor.matmul(gih_ps[:, c:c + 1], whi, grhs[:, 1:2],
                                 start=False, stop=False)
                nc.tensor.matmul(gih_ps[:, c:c + 1],
                                 wih_lo[:, 128 * c:128 * (c + 1)],
                                 grhs[:, 2:3],
                                 start=False, stop=True)
            # fold biases/8 in here: the 8-way fold sum restores them
            pack2 = sp.tile([128, 48], F32, tag="pack2")
            nc.vector.tensor_add(pack2[:], gih_ps[:], bias8[:])

            # ---------------- collective 2: AllGather + fold ----------------
            cc2_in = dp.tile([128, 48], F32, tag="cc2in")
            cc2_out = dp.tile([NCORES, 128, 48], F32, tag="cc2out",
                              addr_space="Shared")
            nc.scalar.dma_start(cc2_in[:], pack2[:])
            nc.gpsimd.collective_compute(AG, BYP, replica_groups=RG,
                                         ins=[cc2_in.opt()],
                                         outs=[cc2_out.opt()])
            gsum3 = sp.tile([128, NCORES, 48], F32, tag="gsum3")
            nc.scalar.dma_start(gsum3[:], cc2_out[:].transpose([1, 0, 2]))
            for j in range(5, 8):
                nc.vector.tensor_copy(outw[j][:, 0:4].bitcast(F32),
                                      gsum3[:, 7, 0:1])
            for j in range(5, 8):
                nc.sync.dma_start(outw[j][:],
                                  out_wt_d[128 * j:128 * (j + 1), :])
            nc.vector.tensor_add(gsum3[:, 0:4, :], gsum3[:, 0:4, :],
                                 gsum3[:, 4:8, :])
            nc.vector.tensor_add(gsum3[:, 0:2, :], gsum3[:, 0:2, :],
                                 gsum3[:, 2:4, :])
            nc.vector.tensor_add(gsum3[:, 0:1, :], gsum3[:, 0:1, :],
                                 gsum3[:, 1:2, :])
            gsum = gsum3[:, 0, :]

            # PE keep-warm during the CC2 wait, then re-warm gated on the
            # CC2 results so the out-projection starts at full clock
            for _ in range(140):
                nc.tensor.matmul(a_psB[:8, 0:1], junkw[:], junkw[:, 0:1],
                                 start=True, stop=True)

            for _ in range(120):
                nc.tensor.matmul(a_psB[:8, 0:1], gsum3[:, 0, 0:8],
                                 ones[:, 0:1], start=True, stop=True)

            # gates (PyTorch order r, z, n); r and z share one Sigmoid pass
            rzpre = sp.tile([128, 16], F32, tag="rzpre")
            nc.vector.tensor_add(rzpre[:], gsum[:, 0:16], gsum[:, 24:40])
            rz_sb = sp.tile([128, 16], F32, tag="rz")
            nc.scalar.activation(rz_sb[:], rzpre[:], ACTF.Sigmoid)
            npre = sp.tile([128, 8], F32, tag="npre")
            nc.vector.tensor_mul(npre[:], rz_sb[:, 0:8], gsum[:, 40:48])
            nc.vector.tensor_add(npre[:], npre[:], gsum[:, 16:24])
            n_sb = sp.tile([128, 8], F32, tag="n")
            nc.scalar.activation(n_sb[:], npre[:], ACTF.Tanh)
            # h_new = n + z * (h0 - n), then scale by 1/S_OUT for the
            # e3m4 out projection (exact power-of-2 shift in bf16)
            hd = sp.tile([128, 8], F32, tag="hd")
            nc.vector.tensor_sub(hd[:], h0cm[:], n_sb[:])
            nc.vector.tensor_mul(hd[:], hd[:], rz_sb[:, 8:16])
            hsum = sp.tile([128, 8], F32, tag="hsum")
            nc.vector.tensor_add(hsum[:], n_sb[:], hd[:])
            hnew_b = sp.tile([128, 8], BF16, tag="hnewb")
            nc.vector.tensor_scalar_mul(hnew_b[:], hsum[:], 1.0 / S_OUT)

            # ---------------- output projection ----------------
            # h-chunk-major so phase j only needs outw[j] (DMA-paced start);
            # DVE accumulates phases into logit_sb.
            logit_sb = sp.tile([128, VT], F32, tag="logit")
            for j in range(8):
                o_ps = pp.tile([128, VT], F32, tag="ops", bufs=2)
                for t in range(VT):
                    nc.tensor.matmul(
                        o_ps[:, t:t + 1],
                        outw[j][:, 128 * t:128 * (t + 1)],
                        hnew_b[:, j:j + 1],
                        start=True, stop=True)
                if j == 0:
                    nc.vector.tensor_add(logit_sb[:], o_ps[:], obias[:])
                else:
                    nc.vector.tensor_add(logit_sb[:], logit_sb[:], o_ps[:])

            nc.gpsimd.dma_start(out_d[:], logit_sb[:])

            if dbg:
                dbg_tiles = {
                    "dbg_a": a_sb, "dbg_exp": exp_sb, "dbg_pack": pack_sb,
                    "dbg_sums": sums, "dbg_attnap": attnap,
                    "dbg_g": g_sb, "dbg_pack2": pack2, "dbg_gsum": gsum,
                    "dbg_hnew": hnew_b,
                }
                for name, t in dbg_tiles.items():
                    shp = list(t[:].shape)
                    d = nc.dram_tensor(name, shp, t[:].dtype,
                                       kind="ExternalOutput")
                    nc.sync.dma_start(d[:], t[:])

    nc.compile()
    _CACHE[key] = nc
    return nc


def _col_major(v, ncols):
    # v [n] -> [128, ncols] with [p, c] = v[128 * c + p]
    return np.ascontiguousarray(v.reshape(ncols, 128).T)


def _pack_rows(a, nb):
    # a [nb*128, w] -> [128, nb*w] with [p, w*i + q] = a[128*i + p, q]
    w = a.shape[1]
    return np.ascontiguousarray(
        a.reshape(nb, 128, w).transpose(1, 0, 2).reshape(128, nb * w))


def _shard(inputs):
    x = np.asarray(inputs["x"]).reshape(-1)
    h0 = np.asarray(inputs["h"], dtype=np.float32).reshape(H)
    enc = np.asarray(inputs["encoder_outputs"], dtype=np.float32)
    emb = np.asarray(inputs["emb"])
    attn_W = np.asarray(inputs["attn_W"], dtype=np.float32)
    attn_b = np.asarray(inputs["attn_b"], dtype=np.float32)
    ctx_W = np.asarray(inputs["ctx_W"], dtype=np.float32)
    ctx_b = np.asarray(inputs["ctx_b"], dtype=np.float32)
    W_ih = np.asarray(inputs["W_ih"], dtype=np.float32)
    W_hh = np.asarray(inputs["W_hh"], dtype=np.float32)
    b_ih = np.asarray(inputs["b_ih"], dtype=np.float32)
    b_hh = np.asarray(inputs["b_hh"], dtype=np.float32)
    out_W = np.asarray(inputs["out_W"], dtype=np.float32)
    out_b = np.asarray(inputs["out_b"], dtype=np.float32)

    xe = np.asarray(emb[int(x[0])], dtype=np.float32)
    catin = np.concatenate([_col_major(xe, 8), _col_major(h0, 8)],
                           axis=1).astype(NPBF16)
    h0cm = _col_major(h0, 8)

    bias8 = np.concatenate([_col_major(b_ih, 24), _col_major(b_hh, 24)],
                           axis=1) / 8.0

    in_maps = []
    for k in range(NCORES):
        lsl = slice(LC * k, LC * (k + 1))
        hsl = slice(HC * k, HC * (k + 1))
        v0, v1 = VC * k, min(VC * (k + 1), V)
        owt = np.zeros((H, VPAD), dtype=NPE3)
        owt[:, :v1 - v0] = (out_W[v0:v1, :].T * S_OUT).astype(NPE3)
        ob = np.zeros(VPAD, dtype=np.float32)
        ob[:v1 - v0] = out_b[v0:v1]

        enc_k = enc[lsl, :]
        enc_hi = enc_k.astype(NPBF16)
        enc_lo = ((enc_k - enc_hi.astype(np.float32)) * S_ENC).astype(NPE3)
        ctxT = np.ascontiguousarray(ctx_W[hsl, :].T)       # [2048, 128]
        ctx_hi = ctxT.astype(NPBF16)
        ctx_lo = ((ctxT[H:] - ctx_hi[H:].astype(np.float32))
                  * S_CTX).astype(NPE3)
        wihT = np.ascontiguousarray(W_ih[:, hsl].T)        # [128, 3072]
        wih_hi = wihT.astype(NPBF16)
        wih_lo = ((wihT - wih_hi.astype(np.float32)) * S_WIH).astype(NPE3)
        whhT = np.ascontiguousarray(W_hh[:, hsl].T)
        whh = (whhT * S_WHH).astype(NPE3)

        in_maps.append({
            "catin": np.ascontiguousarray(catin),
            "catins": np.ascontiguousarray(
                (catin.astype(np.float32) / S_ATTN).astype(NPBF16)),
            "attn_wt": _pack_rows((attn_W[lsl, :].T * S_ATTN).astype(NPE3),
                                  16),
            "attn_b": _col_major(attn_b[lsl], 4),
            "enc_hi": _pack_rows(enc_hi, 4),
            "enc_lo": _pack_rows(enc_lo, 4),
            "ctx_hi": _pack_rows(ctx_hi, 16),
            "ctx_lo": _pack_rows(ctx_lo, 8),
            "ctx_b": ctx_b[hsl].reshape(128, 1).copy(),
            "wih_hi": wih_hi,
            "wih_lo": wih_lo,
            "whh": whh,
            "h0c": (h0[hsl] / S_WHH).reshape(128, 1).astype(NPBF16),
            "h0cm": h0cm,
            "bias8": np.ascontiguousarray(bias8, dtype=np.float32),
            "out_wt": owt,
            "out_b": _col_major(ob, VT),
        })
    return in_maps


def _gather(results):
    logits = np.empty(NCORES * VC, dtype=np.float32)
    for k in range(NCORES):
        chunk = np.asarray(results[k]["out"]).T.ravel()   # [VT*128]
        logits[VC * k:VC * (k + 1)] = chunk[:VC]
    return logits[:V].reshape(1, V)


def kernel(**inputs):
    nc = _build()
    in_maps = _shard(inputs)
    try:
        res = run_bass_kernel_spmd(nc, in_maps, core_ids=list(range(NCORES)))
    except Exception:
        # A dirty device state from a previous process occasionally fails
        # the first launch (NRT_EXEC_UNIT_UNRECOVERABLE); one retry clears.
        res = run_bass_kernel_spmd(nc, in_maps, core_ids=list(range(NCORES)))
    return _gather(res.results)


def kernel_traced(**inputs):
    """Like kernel() but profiles on HW; returns (output, exec_time_ns)."""
    nc = _build()
    in_maps = _shard(inputs)
    res = run_bass_kernel_spmd(nc, in_maps, core_ids=list(range(NCORES)),
                               trace=True)
    return _gather(res.results), res.exec_time_ns


def kernel_debug(**inputs):
    """Run the debug build; returns per-core dicts of all outputs."""
    nc = _build(dbg=True)
    in_maps = _shard(inputs)
    res = run_bass_kernel_spmd(nc, in_maps, core_ids=list(range(NCORES)))
    return res.results


# revision 15
# speedup vs baseline: 1.0811x; 1.0720x over previous
"""Trainium2 Bass kernel for a single-step GRU attention decoder.

Math (matches the reference nn.Module):
    xe  = emb[x]                                   # [H]
    a   = log_softmax(cat(xe, h0) @ attn_W.T + attn_b)   # [L]
    ap  = a @ encoder_outputs                      # [H]
    g   = relu(cat(xe, ap) @ ctx_W.T + ctx_b)      # [H]
    GRU(g, h0) -> h_new                            # [H]
    logits = h_new @ out_W.T + out_b               # [V]

Distribution across 8 NeuronCores (one TRN2 chip):
  - attention sharded over L (512 rows/core); exploiting linearity,
    log_softmax @ enc == a @ enc - (log sum exp a) * colsum(enc), so one
    collective of per-core partials {a@enc, colsum(enc), sum(exp a)} lets
    every core reconstruct attn_applied locally.
  - ctx projection sharded by output rows; GRU mats sharded by *input*
    columns so a single collective of partial (gi, gh) lets every core
    compute the full gates / h_new locally.
  - out projection sharded over vocab (6283 rows/core, padded to 6400).

Both collectives are AllGathers (≈4.6us floor vs ≈10us for AllReduce on
8 cores) followed by a 3-level on-chip fold (log2(8) tensor_adds).

Precision: bulk weights are bf16; "lo" residual terms (enc, ctx ap-part,
W_ih) and the *whole* of W_hh / out_W are fp8 e3m4 with power-of-2
scales folded into the matmul rhs (exact in bf16), halving out_W HBM
traffic.  fp32 accumulation in PSUM throughout; gates/softmax in fp32.
End-to-end rel err ~1.2e-2 (sim) vs the 2e-2 gate.

The PE idles during collective waits; dummy matmuls are issued in those
windows to keep the HAM clock-gate warm (PE at 2.4GHz, not 1.2GHz).
"""

import ml_dtypes
import numpy as np

import concourse.bass as bass
import concourse.bacc as bacc
import concourse.tile as tile
from concourse import mybir
from concourse.bass_utils import run_bass_kernel_spmd

H = 1024
V = 50257
L = 4096
NCORES = 8
LC = L // NCORES          # 512 encoder rows per core
HC = H // NCORES          # 128 hidden chunk per core
VC = -(-V // NCORES)      # 6283 vocab rows per core
VT = 50                   # vocab tiles of 128 per core
VPAD = VT * 128           # 6400
F32 = mybir.dt.float32
BF16 = mybir.dt.bfloat16
E3 = mybir.dt.float8e3
NPBF16 = ml_dtypes.bfloat16
NPE3 = ml_dtypes.float8_e3m4
RG = [list(range(NCORES))]

# power-of-2 scales for the e3m4 tensors (folded into rhs; exact in bf16)
S_ENC = 512.0
S_CTX = 2048.0
S_WIH = 2048.0
S_WHH = 256.0
S_OUT = 256.0
S_ATTN = 256.0

_CACHE = {}


def _build(dbg=False):
    key = ("nc", dbg)
    if key in _CACHE:
        return _CACHE[key]

    nc = bacc.Bacc("TRN2", target_bir_lowering=False, debug=False,
                   num_devices=NCORES)

    def din(name, shape, dt=F32):
        return nc.dram_tensor(name, shape, dt, kind="ExternalInput")

    catin_d = din("catin", [128, 16], BF16)    # cols 0-7 xe, 8-15 h0
    catins_d = din("catins", [128, 16], BF16)  # catin / S_ATTN (for e3 attn)
    attn_wt_d = din("attn_wt", [128, 16 * LC], E3)
    attn_b_d = din("attn_b", [128, 4])
    enc_hi_d = din("enc_hi", [128, 4 * H], BF16)
    enc_lo_d = din("enc_lo", [128, 4 * H], E3)
    ctx_hi_d = din("ctx_hi", [128, 2048], BF16)
    ctx_lo_d = din("ctx_lo", [128, 1024], E3)
    ctx_b_d = din("ctx_b", [128, 1])
    wih_hi_d = din("wih_hi", [HC, 3 * H], BF16)
    wih_lo_d = din("wih_lo", [HC, 3 * H], E3)
    whh_d = din("whh", [HC, 3 * H], E3)
    h0c_d = din("h0c", [128, 1], BF16)         # h0 chunk k / S_WHH
    h0cm_d = din("h0cm", [128, 8])             # full h0, col-major, fp32
    bias8_d = din("bias8", [128, 48])          # cat(b_ih, b_hh)/8 col-major
    out_wt_d = din("out_wt", [H, VPAD], E3)    # out_W vocab chunk, transposed
    out_b_d = din("out_b", [128, VT])
    out_d = nc.dram_tensor("out", [128, VT], F32, kind="ExternalOutput")

    AG = "AllGather"
    BYP = mybir.AluOpType.bypass
    ACTF = mybir.ActivationFunctionType

    with tile.TileContext(nc) as tc:
        with (
            tc.tile_pool(name="wp", bufs=1) as wp,
            tc.tile_pool(name="sp", bufs=1) as sp,
            tc.tile_pool(name="pp", bufs=1, space="PSUM") as pp,
            tc.tile_pool(name="dp", bufs=1, space="DRAM") as dp,
        ):
            # ------------- loads (issue order = priority order) -------------
            # small tiles go on the scalar HWDGE ring (qAct) so the sync
            # ring (qSP) starts streaming the big prefix immediately; both
            # rings' DMAs complete early so no sem-lane entanglement.
            catin = sp.tile([128, 16], BF16, tag="catin")
            nc.scalar.dma_start(catin[:], catin_d[:])
            catins = sp.tile([128, 16], BF16, tag="catins")
            nc.scalar.dma_start(catins[:], catins_d[:])
            abias = sp.tile([128, 4], F32, tag="abias")
            nc.scalar.dma_start(abias[:], attn_b_d[:])
            cbias = sp.tile([128, 1], F32, tag="cbias")
            nc.scalar.dma_start(cbias[:], ctx_b_d[:])
            h0c = sp.tile([128, 1], BF16, tag="h0c")
            nc.scalar.dma_start(h0c[:], h0c_d[:])
            h0cm = sp.tile([128, 8], F32, tag="h0cm")
            nc.scalar.dma_start(h0cm[:], h0cm_d[:])
            bias8 = sp.tile([128, 48], F32, tag="bias8")
            nc.scalar.dma_start(bias8[:], bias8_d[:])
            obias = sp.tile([128, VT], F32, tag="obias")
            nc.scalar.dma_start(obias[:], out_b_d[:])

            attn_sb = wp.tile([128, 16 * LC], E3, tag="attn")
            nc.sync.dma_start(attn_sb[:, :8 * LC], attn_wt_d[:, :8 * LC])
            nc.sync.dma_start(attn_sb[:, 8 * LC:], attn_wt_d[:, 8 * LC:])
            enc_hi = wp.tile([128, 4 * H], BF16, tag="enchi")
            nc.sync.dma_start(enc_hi[:], enc_hi_d[:])
            enc_lo = wp.tile([128, 4 * H], E3, tag="enclo")
            nc.sync.dma_start(enc_lo[:], enc_lo_d[:])
            ctx_hi = wp.tile([128, 2048], BF16, tag="ctxhi")
            nc.sync.dma_start(ctx_hi[:], ctx_hi_d[:])
            ctx_lo = wp.tile([128, 1024], E3, tag="ctxlo")
            nc.sync.dma_start(ctx_lo[:], ctx_lo_d[:])
            wih_hi = wp.tile([128, 3 * H], BF16, tag="wihhi")
            nc.sync.dma_start(wih_hi[:], wih_hi_d[:])
            wih_lo = wp.tile([128, 3 * H], E3, tag="wihlo")
            nc.sync.dma_start(wih_lo[:], wih_lo_d[:])
            whh_sb = wp.tile([128, 3 * H], E3, tag="whh")
            nc.sync.dma_start(whh_sb[:], whh_d[:])

            # out_W tiles are allocated here but their DMAs are issued on
            # the gpsimd queue AFTER the CC1 trigger (below): the 6.5MB/core
            # stream would otherwise compete with every core's attention
            # prefix and scatter the collective entry times.
            outw = [wp.tile([128, VPAD], E3, tag=f"outw{j}", name=f"outw{j}")
                    for j in range(8)]

            # ------------- constants (DVE memsets, no DMA deps) -------------
            junkw = sp.tile([128, 8], BF16, tag="junkw")
            nc.vector.memset(junkw[:], 0.0)
            ones = sp.tile([128, 128], F32, tag="ones")
            nc.vector.memset(ones[:], 1.0)
            # rhs5 per l-tile t: cols 5t..5t+4 = [a_hi, a_lo, 1, a_hi/S, 1/S]
            rhs5 = sp.tile([128, 20], BF16, tag="rhs5")
            nc.vector.memset(rhs5[:, 2:20:5], 1.0)
            nc.vector.memset(rhs5[:, 4:20:5], 1.0 / S_ENC)
            pack_sb = sp.tile([128, 18], F32, tag="pack")
            nc.vector.memset(pack_sb[:, 17:18], 0.0)

            # ---------------- PE warmup (HAM clock-gate) ----------------
            # batch 1 runs from ~1us (memset-gated); batch 2 is gated on the
            # catin DMA (~6us) so activity bridges to the attention matmuls.
            junk_ps = pp.tile([8, 1], F32, tag="junkps")
            for _ in range(64):
                nc.tensor.matmul(junk_ps[:], junkw[:], junkw[:, 0:1],
                                 start=True, stop=True)

            # ---------------- attention logits ----------------
            # a[l] for the 512 local l, laid out [128, 4] col-major tiles.
            # NOTE: accumulation groups must be contiguous in program order.
            a_psA = pp.tile([128, 4], F32, tag="apsA")
            for j in range(4):           # l tiles
                for i in range(8):       # xe half of the cat dim
                    nc.tensor.matmul(
                        a_psA[:, j:j + 1],
                        attn_sb[:, LC * i + 128 * j:LC * i + 128 * (j + 1)],
                        catins[:, i:i + 1],
                        start=(i == 0), stop=(i == 7))
            a_psB = pp.tile([128, 4], F32, tag="apsB")
            for j in range(4):           # l tiles
                for i in range(8, 16):   # h0 half
                    nc.tensor.matmul(
                        a_psB[:, j:j + 1],
                        attn_sb[:, LC * i + 128 * j:LC * i + 128 * (j + 1)],
                        catins[:, i:i + 1],
                        start=(i == 8), stop=(i == 15))

            # gh = W_hh @ h0 and the xe half of the ctx projection depend
            # only on inputs — run them while the DVE builds rhs5, and
            # before the CC1 wait.
            gih_ps = pp.tile([128, 48], F32, tag="gihps")
            for c in range(24):
                nc.tensor.matmul(gih_ps[:, 24 + c:25 + c],
                                 whh_sb[:, 128 * c:128 * (c + 1)], h0c[:],
                                 start=True, stop=True)
            g_ps = pp.tile([128, 2], F32, tag="gps")
            for i in range(8):
                nc.tensor.matmul(g_ps[:, 0:1], ctx_hi[:, 128 * i:128 * (i + 1)],
                                 catin[:, i:i + 1],
                                 start=(i == 0), stop=(i == 7))

            # a = psA + psB + bias; exp + row-sum fused via accum_out
            a_half = sp.tile([128, 4], F32, tag="ahalf")
            nc.vector.tensor_add(a_half[:], a_psA[:], abias[:])
            a_sb = sp.tile([128, 4], F32, tag="a")
            nc.vector.tensor_add(a_sb[:], a_psB[:], a_half[:])
            exp_sb = sp.tile([128, 4], F32, tag="expa")
            exp_r = sp.tile([128, 1], F32, tag="expr")
            nc.scalar.activation(exp_sb[:], a_sb[:], ACTF.Exp,
                                 accum_out=exp_r[:])
            # split a into hi/lo bf16 directly into the rhs5 columns
            nc.vector.tensor_copy(rhs5[:, 0:20:5], a_sb[:])
            a_hif = sp.tile([128, 4], F32, tag="ahif")
            nc.vector.tensor_copy(a_hif[:], rhs5[:, 0:20:5])
            nc.vector.tensor_sub(rhs5[:, 1:20:5], a_sb[:], a_hif[:])
            nc.vector.tensor_scalar_mul(rhs5[:, 3:20:5], rhs5[:, 0:20:5],
                                        1.0 / S_ENC)

            # pack psum cols per h-chunk c: 5c+0 ehi*ahi, +1 ehi*alo,
            # +2 ehi*1, +3 elo*ahi/S, +4 elo*1/S; col 40 = sum(exp a)
            # broadcast to all partitions via the ones matmul.
            pack_ps = pp.tile([128, 41], F32, tag="packps")
            for c in range(8):
                for j in range(4):
                    nc.tensor.matmul(
                        pack_ps[:, 5 * c:5 * c + 3],
                        enc_hi[:, H * j + 128 * c:H * j + 128 * (c + 1)],
                        rhs5[:, 5 * j:5 * j + 3],
                        start=(j == 0), stop=(j == 3))
                for j in range(4):
                    nc.tensor.matmul(
                        pack_ps[:, 5 * c + 3:5 * c + 5],
                        enc_lo[:, H * j + 128 * c:H * j + 128 * (c + 1)],
                        rhs5[:, 5 * j + 3:5 * j + 5],
                        start=(j == 0), stop=(j == 3))
            nc.tensor.matmul(pack_ps[:, 40:41], ones[:], exp_r[:],
                             start=True, stop=True)

            # combine: pack[0:8]=P, pack[8:16]=S, pack[16]=s, pack[17]=0
            # (ops can read at most one PSUM input -> copy to SBUF first)
            pk_sb = sp.tile([128, 41], F32, tag="pksb")
            nc.vector.tensor_copy(pk_sb[:], pack_ps[:])
            nc.vector.tensor_add(pack_sb[:, 0:8], pk_sb[:, 0:40:5],
                                 pk_sb[:, 1:40:5])
            nc.vector.tensor_add(pack_sb[:, 0:8], pack_sb[:, 0:8],
                                 pk_sb[:, 3:40:5])
            nc.vector.tensor_add(pack_sb[:, 8:16], pk_sb[:, 2:40:5],
                                 pk_sb[:, 4:40:5])
            nc.vector.tensor_copy(pack_sb[:, 16:17], pk_sb[:, 40:41])
            # stash the xe half of ctx while the PE is free
            g_xe = sp.tile([128, 1], F32, tag="gxe")
            nc.vector.tensor_copy(g_xe[:], g_ps[:, 0:1])

            # ---------------- collective 1: AllGather + fold ----------------
            # all collective-adjacent DMAs ride the gpsimd SWDGE lanes so
            # their (late) completions never block the HWDGE weight-stream
            # semaphore lanes.
            cc1_in = dp.tile([128, 18], F32, tag="cc1in")
            cc1_out = dp.tile([NCORES, 128, 18], F32, tag="cc1out",
                              addr_space="Shared")
            nc.gpsimd.dma_start(cc1_in[:], pack_sb[:])
            nc.gpsimd.collective_compute(AG, BYP, replica_groups=RG,
                                         ins=[cc1_in.opt()],
                                         outs=[cc1_out.opt()])
            sums3 = sp.tile([128, NCORES, 18], F32, tag="sums3")
            nc.gpsimd.dma_start(sums3[:], cc1_out[:].transpose([1, 0, 2]))
            # The collective mesh makes no progress while bulk DMA is in
            # flight, so the 6.5MB/core out_W stream is released in two
            # waves into the collective-free windows: tiles 0-4 after the
            # CC1 result lands (streams during the local ctx/GRU chain),
            # tiles 5-7 after CC2 (streams during the gate math).  The
            # gate is a dummy DVE write into each tile reading a region
            # the folds don't touch, so it depends only on the return DMA.
            for j in range(4):
                nc.vector.tensor_copy(outw[j][:, 0:4].bitcast(F32),
                                      sums3[:, 7, 0:1])
            for j in range(4):
                nc.sync.dma_start(outw[j][:],
                                  out_wt_d[128 * j:128 * (j + 1), :])
            nc.vector.tensor_add(sums3[:, 0:4, :], sums3[:, 0:4, :],
                                 sums3[:, 4:8, :])
            nc.vector.tensor_add(sums3[:, 0:2, :], sums3[:, 0:2, :],
                                 sums3[:, 2:4, :])
            nc.vector.tensor_add(sums3[:, 0:1, :], sums3[:, 0:1, :],
                                 sums3[:, 1:2, :])
            sums = sums3[:, 0, :]

            # c = log(sum exp) — s was broadcast to all partitions pre-CC
            c_sb = sp.tile([128, 1], F32, tag="c")
            nc.scalar.activation(c_sb[:], sums[:, 16:17], ACTF.Ln)
            # attn_applied = P - c * S   [128, 8]
            atmp = sp.tile([128, 8], F32, tag="atmp")
            nc.vector.tensor_scalar(atmp[:], sums[:, 8:16], c_sb[:], None,
                                    mybir.AluOpType.mult)
            attnap = sp.tile([128, 8], F32, tag="attnap")
            nc.vector.tensor_sub(attnap[:], sums[:, 0:8], atmp[:])
            # ctx rhs cols per m: [ap_hi, ap_lo, ap_hi/S_CTX]
            aprhs = sp.tile([128, 24], BF16, tag="aprhs")
            nc.vector.tensor_copy(aprhs[:, 0:24:3], attnap[:])
            ap_hif = sp.tile([128, 8], F32, tag="aphif")
            nc.vector.tensor_copy(ap_hif[:], aprhs[:, 0:24:3])
            nc.vector.tensor_sub(aprhs[:, 1:24:3], attnap[:], ap_hif[:])
            nc.vector.tensor_scalar_mul(aprhs[:, 2:24:3], aprhs[:, 0:24:3],
                                        1.0 / S_CTX)

            # ---------------- context projection (attnap half) ----------
            nmm = 24
            k = 0
            for m in range(8):
                t = ctx_hi[:, 128 * (8 + m):128 * (9 + m)]
                nc.tensor.matmul(g_ps[:, 1:2], t, aprhs[:, 3 * m:3 * m + 1],
                                 start=(k == 0), stop=(k == nmm - 1))
                k += 1
                nc.tensor.matmul(g_ps[:, 1:2], t, aprhs[:, 3 * m + 1:3 * m + 2],
                                 start=False, stop=(k == nmm - 1))
                k += 1
                nc.tensor.matmul(g_ps[:, 1:2], ctx_lo[:, 128 * m:128 * (m + 1)],
                                 aprhs[:, 3 * m + 2:3 * m + 3],
                                 start=False, stop=(k == nmm - 1))
                k += 1
            gpre = sp.tile([128, 1], F32, tag="gpre")
            nc.vector.tensor_add(gpre[:], g_ps[:, 1:2], g_xe[:])
            g_sb = sp.tile([128, 1], F32, tag="g")
            nc.scalar.activation(g_sb[:], gpre[:], ACTF.Relu, bias=cbias[:])
            # GRU rhs cols: [g_hi, g_lo, g_hi/S_WIH]
            grhs = sp.tile([128, 3], BF16, tag="grhs")
            nc.vector.tensor_copy(grhs[:, 0:1], g_sb[:])
            g_hif = sp.tile([128, 1], F32, tag="ghif")
            nc.vector.tensor_copy(g_hif[:], grhs[:, 0:1])
            nc.vector.tensor_sub(grhs[:, 1:2], g_sb[:], g_hif[:])
            nc.vector.tensor_scalar_mul(grhs[:, 2:3], grhs[:, 0:1],
                                        1.0 / S_WIH)

            # ---------------- GRU partial matvecs (gi; gh ran earlier) ----
            for c in range(24):
                whi = wih_hi[:, 128 * c:128 * (c + 1)]
                nc.tensor.matmul(gih_ps[:, c:c + 1], whi, grhs[:, 0:1],
                                 start=True, stop=False)
                nc.tensor.matmul(gih_ps[:, c:c + 1], whi, grhs[:, 1:2],
                                 start=False, stop=False)
                nc.tensor.matmul(gih_ps[:, c:c + 1],
                                 wih_lo[:, 128 * c:128 * (c + 1)],
                                 grhs[:, 2:3],
                                 start=False, stop=True)
            # fold biases/8 in here: the 8-way fold sum restores them
            pack2 = sp.tile([128, 48], F32, tag="pack2")
            nc.vector.tensor_add(pack2[:], gih_ps[:], bias8[:])

            # ---------------- collective 2: AllGather + fold ----------------
            cc2_in = dp.tile([128, 48], F32, tag="cc2in")
            cc2_out = dp.tile([NCORES, 128, 48], F32, tag="cc2out",
                              addr_space="Shared")
            nc.gpsimd.dma_start(cc2_in[:], pack2[:])
            nc.gpsimd.collective_compute(AG, BYP, replica_groups=RG,
                                         ins=[cc2_in.opt()],
                                         outs=[cc2_out.opt()])
            gsum3 = sp.tile([128, NCORES, 48], F32, tag="gsum3")
            nc.gpsimd.dma_start(gsum3[:], cc2_out[:].transpose([1, 0, 2]))
            for j in range(4, 8):
                nc.vector.tensor_copy(outw[j][:, 0:4].bitcast(F32),
                                      gsum3[:, 7, 0:1])
            for j in range(4, 8):
                nc.sync.dma_start(outw[j][:],
                                  out_wt_d[128 * j:128 * (j + 1), :])
            nc.vector.tensor_add(gsum3[:, 0:4, :], gsum3[:, 0:4, :],
                                 gsum3[:, 4:8, :])
            nc.vector.tensor_add(gsum3[:, 0:2, :], gsum3[:, 0:2, :],
                                 gsum3[:, 2:4, :])
            nc.vector.tensor_add(gsum3[:, 0:1, :], gsum3[:, 0:1, :],
                                 gsum3[:, 1:2, :])
            gsum = gsum3[:, 0, :]


            # gates (PyTorch order r, z, n); r and z share one Sigmoid pass
            rzpre = sp.tile([128, 16], F32, tag="rzpre")
            nc.vector.tensor_add(rzpre[:], gsum[:, 0:16], gsum[:, 24:40])
            rz_sb = sp.tile([128, 16], F32, tag="rz")
            nc.scalar.activation(rz_sb[:], rzpre[:], ACTF.Sigmoid)
            npre = sp.tile([128, 8], F32, tag="npre")
            nc.vector.tensor_mul(npre[:], rz_sb[:, 0:8], gsum[:, 40:48])
            nc.vector.tensor_add(npre[:], npre[:], gsum[:, 16:24])
            n_sb = sp.tile([128, 8], F32, tag="n")
            nc.scalar.activation(n_sb[:], npre[:], ACTF.Tanh)
            # h_new = n + z * (h0 - n), then scale by 1/S_OUT for the
            # e3m4 out projection (exact power-of-2 shift in bf16)
            hd = sp.tile([128, 8], F32, tag="hd")
            nc.vector.tensor_sub(hd[:], h0cm[:], n_sb[:])
            nc.vector.tensor_mul(hd[:], hd[:], rz_sb[:, 8:16])
            hsum = sp.tile([128, 8], F32, tag="hsum")
            nc.vector.tensor_add(hsum[:], n_sb[:], hd[:])
            hnew_b = sp.tile([128, 8], BF16, tag="hnewb")
            nc.vector.tensor_scalar_mul(hnew_b[:], hsum[:], 1.0 / S_OUT)

            # ---------------- output projection ----------------
            # h-chunk-major so phase j only needs outw[j] (DMA-paced start);
            # DVE accumulates phases into logit_sb.
            logit_sb = sp.tile([128, VT], F32, tag="logit")
            for j in range(8):
                o_ps = pp.tile([128, VT], F32, tag="ops", bufs=2)
                for t in range(VT):
                    nc.tensor.matmul(
                        o_ps[:, t:t + 1],
                        outw[j][:, 128 * t:128 * (t + 1)],
                        hnew_b[:, j:j + 1],
                        start=True, stop=True)
                if j == 0:
                    nc.vector.tensor_add(logit_sb[:], o_ps[:], obias[:])
                else:
                    nc.vector.tensor_add(logit_sb[:], logit_sb[:], o_ps[:])

            nc.gpsimd.dma_start(out_d[:], logit_sb[:])

            if dbg:
                dbg_tiles = {
                    "dbg_a": a_sb, "dbg_exp": exp_sb, "dbg_pack": pack_sb,
                    "dbg_sums": sums, "dbg_attnap": attnap,
                    "dbg_g": g_sb, "dbg_pack2": pack2, "dbg_gsum": gsum,
                    "dbg_hnew": hnew_b,
                }
                for name, t in dbg_tiles.items():
                    shp = list(t[:].shape)
                    d = nc.dram_tensor(name, shp, t[:].dtype,
                                       kind="ExternalOutput")
                    nc.sync.dma_start(d[:], t[:])

    nc.compile()
    _CACHE[key] = nc
    return nc


def _col_major(v, ncols):
    # v [n] -> [128, ncols] with [p, c] = v[128 * c + p]
    return np.ascontiguousarray(v.reshape(ncols, 128).T)


def _pack_rows(a, nb):
    # a [nb*128, w] -> [128, nb*w] with [p, w*i + q] = a[128*i + p, q]
    w = a.shape[1]
    return np.ascontiguousarray(
        a.reshape(nb, 128, w).transpose(1, 0, 2).reshape(128, nb * w))


def _shard(inputs):
    x = np.asarray(inputs["x"]).reshape(-1)
    h0 = np.asarray(inputs["h"], dtype=np.float32).reshape(H)
    enc = np.asarray(inputs["encoder_outputs"], dtype=np.float32)
    emb = np.asarray(inputs["emb"])
    attn_W = np.asarray(inputs["attn_W"], dtype=np.float32)
    attn_b = np.asarray(inputs["attn_b"], dtype=np.float32)
    ctx_W = np.asarray(inputs["ctx_W"], dtype=np.float32)
    ctx_b = np.asarray(inputs["ctx_b"], dtype=np.float32)
    W_ih = np.asarray(inputs["W_ih"], dtype=np.float32)
    W_hh = np.asarray(inputs["W_hh"], dtype=np.float32)
    b_ih = np.asarray(inputs["b_ih"], dtype=np.float32)
    b_hh = np.asarray(inputs["b_hh"], dtype=np.float32)
    out_W = np.asarray(inputs["out_W"], dtype=np.float32)
    out_b = np.asarray(inputs["out_b"], dtype=np.float32)

    xe = np.asarray(emb[int(x[0])], dtype=np.float32)
    catin = np.concatenate([_col_major(xe, 8), _col_major(h0, 8)],
                           axis=1).astype(NPBF16)
    h0cm = _col_major(h0, 8)

    bias8 = np.concatenate([_col_major(b_ih, 24), _col_major(b_hh, 24)],
                           axis=1) / 8.0

    in_maps = []
    for k in range(NCORES):
        lsl = slice(LC * k, LC * (k + 1))
        hsl = slice(HC * k, HC * (k + 1))
        v0, v1 = VC * k, min(VC * (k + 1), V)
        owt = np.zeros((H, VPAD), dtype=NPE3)
        owt[:, :v1 - v0] = (out_W[v0:v1, :].T * S_OUT).astype(NPE3)
        ob = np.zeros(VPAD, dtype=np.float32)
        ob[:v1 - v0] = out_b[v0:v1]

        enc_k = enc[lsl, :]
        enc_hi = enc_k.astype(NPBF16)
        enc_lo = ((enc_k - enc_hi.astype(np.float32)) * S_ENC).astype(NPE3)
        ctxT = np.ascontiguousarray(ctx_W[hsl, :].T)       # [2048, 128]
        ctx_hi = ctxT.astype(NPBF16)
        ctx_lo = ((ctxT[H:] - ctx_hi[H:].astype(np.float32))
                  * S_CTX).astype(NPE3)
        wihT = np.ascontiguousarray(W_ih[:, hsl].T)        # [128, 3072]
        wih_hi = wihT.astype(NPBF16)
        wih_lo = ((wihT - wih_hi.astype(np.float32)) * S_WIH).astype(NPE3)
        whhT = np.ascontiguousarray(W_hh[:, hsl].T)
        whh = (whhT * S_WHH).astype(NPE3)

        in_maps.append({
            "catin": np.ascontiguousarray(catin),
            "catins": np.ascontiguousarray(
                (catin.astype(np.float32) / S_ATTN).astype(NPBF16)),
            "attn_wt": _pack_rows((attn_W[lsl, :].T * S_ATTN).astype(NPE3),
                                  16),
            "attn_b": _col_major(attn_b[lsl], 4),
            "enc_hi": _pack_rows(enc_hi, 4),
            "enc_lo": _pack_rows(enc_lo, 4),
            "ctx_hi": _pack_rows(ctx_hi, 16),
            "ctx_lo": _pack_rows(ctx_lo, 8),
            "ctx_b": ctx_b[hsl].reshape(128, 1).copy(),
            "wih_hi": wih_hi,
            "wih_lo": wih_lo,
            "whh": whh,
            "h0c": (h0[hsl] / S_WHH).reshape(128, 1).astype(NPBF16),
            "h0cm": h0cm,
            "bias8": np.ascontiguousarray(bias8, dtype=np.float32),
            "out_wt": owt,
            "out_b": _col_major(ob, VT),
        })
    return in_maps


def _gather(results):
    logits = np.empty(NCORES * VC, dtype=np.float32)
    for k in range(NCORES):
        chunk = np.asarray(results[k]["out"]).T.ravel()   # [VT*128]
        logits[VC * k:VC * (k + 1)] = chunk[:VC]
    return logits[:V].reshape(1, V)


def kernel(**inputs):
    nc = _build()
    in_maps = _shard(inputs)
    try:
        res = run_bass_kernel_spmd(nc, in_maps, core_ids=list(range(NCORES)))
    except Exception:
        # A dirty device state from a previous process occasionally fails
        # the first launch (NRT_EXEC_UNIT_UNRECOVERABLE); one retry clears.
        res = run_bass_kernel_spmd(nc, in_maps, core_ids=list(range(NCORES)))
    return _gather(res.results)


def kernel_traced(**inputs):
    """Like kernel() but profiles on HW; returns (output, exec_time_ns)."""
    nc = _build()
    in_maps = _shard(inputs)
    res = run_bass_kernel_spmd(nc, in_maps, core_ids=list(range(NCORES)),
                               trace=True)
    return _gather(res.results), res.exec_time_ns


def kernel_debug(**inputs):
    """Run the debug build; returns per-core dicts of all outputs."""
    nc = _build(dbg=True)
    in_maps = _shard(inputs)
    res = run_bass_kernel_spmd(nc, in_maps, core_ids=list(range(NCORES)))
    return res.results
